# revision 3
# baseline (speedup 1.0000x reference)
"""Trainium2 Bass kernel for nn_Network_81862076662591 (sampling network).

Self-contained: takes FULL inputs (as produced by the problem's
setup_inputs), data-parallel shards batch B=256 over 8 NeuronCores
(32 rows each, per-iteration weights replicated), runs the fused
LSTM + gumbel-argmax sampling + MLP scan on-device, and returns the
full [256, 10, 100] output.

Kernel design (per core, B=32, T=100, D=784=7x112, G=256, H=128):
  - activations kept feature-major [feat, batch] so weight-stationary
    fp32 matmuls chain without transposes;
  - sampling (argmax over D) runs batch-major via DVE max/max_index;
    the one-hot is transposed back feature-major with 7 PE transposes;
  - st = hard + soft - stop_grad(soft) == hard numerically, so the
    softmax is skipped entirely;
  - mem is tracked as mask^T (feature-major) and -1e9*mem (batch-major,
    added to the gumbel-perturbed logits for masking);
  - all biases in this problem are zeros: bg2 is folded into the gumbel
    tensor host-side; the remaining zero biases are dropped.
"""
from contextlib import ExitStack

import numpy as np

import concourse.bass as bass
import concourse.mybir as mybir
import concourse.tile as tile
from concourse.vector_clock import ScopedClock
from concourse.bass_utils import run_bass_kernel_spmd

F32 = mybir.dt.float32
ALU = mybir.AluOpType
ACTF = mybir.ActivationFunctionType

NCORES = 8
B = 32          # per-core batch
D = 784
KP = 112        # feature tile partition size (784 = 7*112)
NK = 7
G = 256
H = 128
T = 100
NEGBIG = -1.0e9


class _TileContextSplitDrain(tile.TileContext):
    """This walrus build rejects >1 sem-wait on the kernel-tail Drain;
    split the accumulated waits across several sequential drains."""

    def _drain_and_barrier(self, tick_clock, wait_clock):
        drain_inst = self.nc.sync.drain()
        wait_clock.add_sem_waits(
            drain_inst.ins, ScopedClock({None: tick_clock.global_clock}))
        si = drain_inst.ins.sync_info
        waits = list(si.on_wait or []) if si is not None else []
        if len(waits) > 1:
            si.on_wait = [waits[0]]
            for w in waits[1:]:
                d2 = self.nc.sync.drain()
                if d2.ins.sync_info is None:
                    d2.ins.sync_info = mybir.SyncInfo(on_wait=[w], on_update=[])
                else:
                    d2.ins.sync_info.on_wait = [w]
        self.nc.all_engine_barrier()
        assert self.sems is not None
        popped = self.nc._tile_sem_poison_stack.pop()
        assert popped is self._sem_poison
        self.nc.clear_and_free_semaphores(list(self.sems.allocated().values()))
        self.nc.all_engine_barrier()


def _split_multi_waits(nc, limit=1):
    """This walrus accepts only `limit` sem-waits per instruction; move the
    excess onto same-engine sequencer NOPs inserted immediately before."""
    import copy

    proto = nc.vector.isa(nc.isa.Opcode.NEURON_ISA_TPB_OPCODE_NOP, {}).ins
    nop_ctr = [0]

    def make_nop(engine, waits):
        nop = copy.deepcopy(proto)
        nop_ctr[0] += 1
        nop.name = f"waitnop-{nop_ctr[0]}"
        nop.engine = engine
        nop.sync_info = mybir.SyncInfo(on_wait=list(waits), on_update=[])
        return nop

    skip = ("InstAllEngineBarrier", "InstEventSemaphore")
    for fn in nc.m.functions:
        for bb in fn.blocks:
            insts = bb.instructions
            if insts and insts[-1] is proto:
                insts.pop()
            out = []
            for inst in insts:
                si = inst.sync_info
                waits = list(si.on_wait or []) if si is not None else []
                if len(waits) > limit and type(inst).__name__ not in skip:
                    for i in range(0, len(waits) - limit, limit):
                        out.append(make_nop(inst.engine, waits[i:i + limit]))
                    si.on_wait = waits[len(waits) - limit:]
                out.append(inst)
            bb.instructions[:] = out


def _declare_params(nc):
    dp = nc.declare_dram_parameter
    p = {}
    p["x"] = dp("x", [B, D], F32, isOutput=False)
    p["gum"] = dp("gum", [T, B, D], F32, isOutput=False)
    p["W1"] = dp("W1", [T, D, D], F32, isOutput=False)
    p["W2"] = dp("W2", [T, D, G], F32, isOutput=False)
    p["W3"] = dp("W3", [T, G, H], F32, isOutput=False)
    p["W4"] = dp("W4", [T, H, H], F32, isOutput=False)
    p["Wf2"] = dp("Wf2", [T, H, 10], F32, isOutput=False)
    p["Wg1"] = dp("Wg1", [T, H, G], F32, isOutput=False)
    p["Wg2"] = dp("Wg2", [T, G, D], F32, isOutput=False)
    p["Wih"] = dp("Wih", [H, 4 * H], F32, isOutput=False)
    p["Whh"] = dp("Whh", [H, 4 * H], F32, isOutput=False)
    p["ident"] = dp("ident", [B, B], F32, isOutput=False)
    p["iota"] = dp("iota", [B, D], F32, isOutput=False)
    p["out"] = dp("out", [B, 10 * T], F32, isOutput=True)
    return p


def _build(ctx, tc, p, w_bufs=2):
    nc = tc.nc

    const_pool = ctx.enter_context(tc.tile_pool(name="const", bufs=1))
    state_pool = ctx.enter_context(tc.tile_pool(name="state", bufs=1))
    wpool = ctx.enter_context(tc.tile_pool(name="w", bufs=w_bufs))
    spool = ctx.enter_context(tc.tile_pool(name="samp", bufs=2))
    psum = ctx.enter_context(tc.tile_pool(name="ps", bufs=1, space="PSUM"))

    WihS = const_pool.tile([H, 4 * H], F32, tag="wih")
    WhhS = const_pool.tile([H, 4 * H], F32, tag="whh")
    IDENT = const_pool.tile([B, B], F32, tag="ident")
    IOTA = const_pool.tile([B, D], F32, tag="iota")
    XB = const_pool.tile([B, D], F32, tag="xb")
    nc.sync.dma_start(WihS[:], p["Wih"].ap())
    nc.sync.dma_start(WhhS[:], p["Whh"].ap())
    nc.sync.dma_start(IDENT[:], p["ident"].ap())
    nc.sync.dma_start(IOTA[:], p["iota"].ap())
    nc.sync.dma_start(XB[:], p["x"].ap())

    A_fm = state_pool.tile([H, B], F32, tag="a")      # lin^T
    H_fm = state_pool.tile([H, B], F32, tag="h")
    C_fm = state_pool.tile([H, B], F32, tag="c")
    MEMFM = state_pool.tile([KP, NK, B], F32, tag="memfm")   # mask^T
    NEGMEM = state_pool.tile([B, D], F32, tag="negmem")      # -1e9 * mem
    XFM = state_pool.tile([KP, NK, B], F32, tag="xfm")       # x^T
    YT = state_pool.tile([KP, NK, B], F32, tag="yt")         # (mask*x)^T
    A1 = state_pool.tile([KP, NK, B], F32, tag="a1")
    A2 = state_pool.tile([H, 2, B], F32, tag="a2")
    SC1 = state_pool.tile([KP, NK, B], F32, tag="sc1")
    SC2 = state_pool.tile([H, 2, B], F32, tag="sc2")
    SC3 = state_pool.tile([H, B], F32, tag="sc3")
    A3 = state_pool.tile([H, B], F32, tag="a3")
    G1S = state_pool.tile([H, 2, B], F32, tag="g1")
    SI = state_pool.tile([H, B], F32, tag="si")
    SF = state_pool.tile([H, B], F32, tag="sf")
    SO = state_pool.tile([H, B], F32, tag="so")
    TG = state_pool.tile([H, B], F32, tag="tg")
    TC = state_pool.tile([H, B], F32, tag="tc")
    U0 = state_pool.tile([H, B], F32, tag="u0")
    U1 = state_pool.tile([H, B], F32, tag="u1")
    SOUT = state_pool.tile([B, 10, T], F32, tag="sout")

    nc.vector.memset(A_fm[:], 0.0)
    nc.vector.memset(H_fm[:], 0.0)
    nc.vector.memset(C_fm[:], 0.0)
    nc.vector.memset(MEMFM[:], 0.0)
    nc.vector.memset(NEGMEM[:], 0.0)

    ps_x = psum.tile([KP, NK, B], F32, tag="y1")
    for k in range(NK):
        nc.tensor.transpose(ps_x[:, k, :], XB[:, k * KP:(k + 1) * KP], IDENT[:])
    nc.scalar.copy(XFM[:], ps_x[:])

    for t in range(T):
        # weight loads for iteration t
        W1S = wpool.tile([KP, NK, D], F32, tag="w1")
        W2S = wpool.tile([KP, NK, G], F32, tag="w2")
        W3S = wpool.tile([H, 2, H], F32, tag="w3")
        W4S = wpool.tile([H, H], F32, tag="w4")
        Wf2S = wpool.tile([H, 10], F32, tag="wf2")
        Wg1S = wpool.tile([H, G], F32, tag="wg1")
        Wg2S = wpool.tile([H, 2, D], F32, tag="wg2")
        GUM = wpool.tile([B, D], F32, tag="gum")
        nc.sync.dma_start(
            W1S[:], p["W1"].ap()[t].rearrange("(k p) n -> p k n", p=KP))
        nc.sync.dma_start(
            W2S[:], p["W2"].ap()[t].rearrange("(k p) n -> p k n", p=KP))
        nc.sync.dma_start(
            W3S[:], p["W3"].ap()[t].rearrange("(k p) n -> p k n", p=H))
        nc.sync.dma_start(W4S[:], p["W4"].ap()[t])
        nc.sync.dma_start(Wf2S[:], p["Wf2"].ap()[t])
        nc.sync.dma_start(Wg1S[:], p["Wg1"].ap()[t])
        nc.sync.dma_start(
            Wg2S[:], p["Wg2"].ap()[t].rearrange("(k p) n -> p k n", p=H))
        nc.sync.dma_start(GUM[:], p["gum"].ap()[t])

        # LSTM cell
        ps_g = psum.tile([H, 4, B], F32, tag="g")
        for j in range(4):
            nc.tensor.matmul(ps_g[:, j, :], WihS[:, j * H:(j + 1) * H],
                             A_fm[:], start=True, stop=False)
            nc.tensor.matmul(ps_g[:, j, :], WhhS[:, j * H:(j + 1) * H],
                             H_fm[:], start=False, stop=True)
        nc.scalar.activation(SI[:], ps_g[:, 0, :], ACTF.Sigmoid)
        nc.scalar.activation(SF[:], ps_g[:, 1, :], ACTF.Sigmoid)
        nc.scalar.activation(TG[:], ps_g[:, 2, :], ACTF.Tanh)
        nc.scalar.activation(SO[:], ps_g[:, 3, :], ACTF.Sigmoid)
        nc.vector.tensor_tensor(U0[:], SF[:], C_fm[:], ALU.mult)
        nc.vector.tensor_tensor(U1[:], SI[:], TG[:], ALU.mult)
        nc.vector.tensor_tensor(C_fm[:], U0[:], U1[:], ALU.add)
        nc.scalar.activation(TC[:], C_fm[:], ACTF.Tanh)
        nc.vector.tensor_tensor(H_fm[:], SO[:], TC[:], ALU.mult)

        # gating MLP -> logits (batch-major)
        ps_g1 = psum.tile([H, 2, B], F32, tag="g1")
        for m in range(2):
            nc.tensor.matmul(ps_g1[:, m, :], Wg1S[:, m * H:(m + 1) * H],
                             H_fm[:], start=True, stop=True)
        nc.scalar.copy(SC2[:], ps_g1[:])
        nc.vector.scalar_tensor_tensor(G1S[:], SC2[:], 0.2, SC2[:],
                                       ALU.mult, ALU.max)

        ps_la = psum.tile([B, 512], F32, tag="la")
        ps_lb = psum.tile([B, D - 512], F32, tag="lb")
        for k in range(2):
            nc.tensor.matmul(ps_la[:], G1S[:, k, :], Wg2S[:, k, 0:512],
                             start=(k == 0), stop=(k == 1))
        for k in range(2):
            nc.tensor.matmul(ps_lb[:], G1S[:, k, :], Wg2S[:, k, 512:D],
                             start=(k == 0), stop=(k == 1))

        # sampling
        GMEM = spool.tile([B, D], F32, tag="gmem")
        PERT = spool.tile([B, D], F32, tag="pert")
        MAX8 = spool.tile([B, 8], F32, tag="max8")
        IDX8 = spool.tile([B, 8], mybir.dt.uint32, tag="idx8")
        IDXF = spool.tile([B, 1], F32, tag="idxf")
        HARD = spool.tile([B, D], F32, tag="hard")
        NEGHARD = spool.tile([B, D], F32, tag="neghard")

        nc.gpsimd.tensor_tensor(GMEM[:], GUM[:], NEGMEM[:], ALU.add)
        nc.vector.tensor_tensor(PERT[:, 0:512], ps_la[:], GMEM[:, 0:512],
                                ALU.add)
        nc.vector.tensor_tensor(PERT[:, 512:D], ps_lb[:], GMEM[:, 512:D],
                                ALU.add)
        nc.vector.max(MAX8[:], PERT[:])
        nc.vector.max_index(IDX8[:], MAX8[:], PERT[:])
        nc.vector.tensor_copy(IDXF[:], IDX8[:, 0:1])
        nc.vector.tensor_scalar(HARD[:], IOTA[:], IDXF[:], None, ALU.is_equal)
        nc.vector.tensor_scalar(NEGHARD[:], IOTA[:], IDXF[:], NEGBIG,
                                ALU.is_equal, ALU.mult)
        nc.gpsimd.tensor_tensor(NEGMEM[:], NEGHARD[:], NEGMEM[:], ALU.add)
        # transpose one-hot to fm, update mask^T, y^T = mask^T * x^T
        ps_tr = psum.tile([KP, NK, B], F32, tag="y1")
        for k in range(NK):
            nc.tensor.transpose(ps_tr[:, k, :], HARD[:, k * KP:(k + 1) * KP],
                                IDENT[:])
        nc.vector.tensor_tensor(MEMFM[:], ps_tr[:], MEMFM[:], ALU.add)
        nc.vector.tensor_tensor(YT[:], MEMFM[:], XFM[:], ALU.mult)

        # f1 MLP
        ps_y1 = psum.tile([KP, NK, B], F32, tag="y1")
        for m in range(NK):
            for k in range(NK):
                nc.tensor.matmul(ps_y1[:, m, :],
                                 W1S[:, k, m * KP:(m + 1) * KP],
                                 YT[:, k, :], start=(k == 0), stop=(k == NK - 1))
        nc.scalar.copy(SC1[:], ps_y1[:])
        nc.vector.scalar_tensor_tensor(A1[:], SC1[:], 0.2, SC1[:],
                                       ALU.mult, ALU.max)

        ps_a2 = psum.tile([H, 2, B], F32, tag="a2")
        for m in range(2):
            for k in range(NK):
                nc.tensor.matmul(ps_a2[:, m, :],
                                 W2S[:, k, m * H:(m + 1) * H],
                                 A1[:, k, :], start=(k == 0), stop=(k == NK - 1))
        nc.scalar.copy(SC2[:], ps_a2[:])
        nc.vector.scalar_tensor_tensor(A2[:], SC2[:], 0.2, SC2[:],
                                       ALU.mult, ALU.max)

        ps_sm = psum.tile([H, 96], F32, tag="sm")
        for k in range(2):
            nc.tensor.matmul(ps_sm[:, 0:B], W3S[:, k, :], A2[:, k, :],
                             start=(k == 0), stop=(k == 1))
        nc.scalar.copy(SC3[:], ps_sm[:, 0:B])
        nc.vector.scalar_tensor_tensor(A3[:], SC3[:], 0.2, SC3[:],
                                       ALU.mult, ALU.max)

        nc.tensor.matmul(ps_sm[:, B:2 * B], W4S[:], A3[:],
                         start=True, stop=True)
        nc.scalar.copy(SC3[:], ps_sm[:, B:2 * B])
        nc.vector.scalar_tensor_tensor(A_fm[:], SC3[:], 0.2, SC3[:],
                                       ALU.mult, ALU.max)

        nc.tensor.matmul(ps_sm[0:B, 2 * B:2 * B + 10], A_fm[:], Wf2S[:],
                         start=True, stop=True)
        nc.scalar.copy(SOUT[:, :, t], ps_sm[0:B, 2 * B:2 * B + 10])

    nc.sync.dma_start(p["out"].ap(), SOUT[:].rearrange("b c t -> b (c t)"))


_CACHE = {}
TRACE = False
LAST_RES = None


def _get_nc(w_bufs=2):
    key = ("nc", w_bufs)
    if key not in _CACHE:
        nc = bass.Bass("TRN2", target_bir_lowering=False, debug=False)
        p = _declare_params(nc)
        with _TileContextSplitDrain(nc) as tc:
            with ExitStack() as ctx:
                _build(ctx, tc, p, w_bufs=w_bufs)
        _split_multi_waits(nc)
        _CACHE[key] = nc
    return _CACHE[key]


def kernel(**inputs) -> np.ndarray:
    f = lambda k: np.ascontiguousarray(np.asarray(inputs[k]), dtype=np.float32)
    x = f("x")
    gumbel = f("gumbel")
    bg2 = f("bg2")
    gum_all = gumbel + bg2[:, None, :]          # fold bg2 into the noise
    # remaining biases are zeros in this problem; verify cheaply
    for bn in ("b1", "b2", "b3", "b4", "bf2", "bg1", "bih", "bhh"):
        if bn in inputs and np.any(np.asarray(inputs[bn])):
            raise NotImplementedError(f"nonzero bias {bn} not supported")

    shared = {
        "Wih": f("Wih"), "Whh": f("Whh"),
        "ident": np.eye(B, dtype=np.float32),
        "iota": np.tile(np.arange(D, dtype=np.float32), (B, 1)),
    }
    for k in ("W1", "W2", "W3", "W4", "Wf2", "Wg1", "Wg2"):
        shared[k] = f(k)

    in_maps = []
    for c in range(NCORES):
        sl = slice(c * B, (c + 1) * B)
        m = dict(shared)
        m["x"] = np.ascontiguousarray(x[sl])
        m["gum"] = np.ascontiguousarray(gum_all[:, sl])
        in_maps.append(m)

    nc = _get_nc()
    global LAST_RES
    res = run_bass_kernel_spmd(nc, in_maps, list(range(NCORES)), trace=TRACE)
    LAST_RES = res
    out = np.concatenate(
        [res.results[c]["out"].reshape(B, 10, T) for c in range(NCORES)],
        axis=0)
    return out.astype(np.float32)



# revision 42
# speedup vs baseline: 3.7540x; 3.7540x over previous
"""Trainium2 Bass kernel for nn_Network_81862076662591 (sampling network).

Self-contained: takes FULL inputs (as produced by the problem's
setup_inputs), data-parallel shards batch B=256 over 8 NeuronCores
(32 rows each, per-iteration weights replicated), runs the fused
LSTM + gumbel-argmax sampling + MLP scan on-device, and returns the
full [256, 10, 100] output.

Design notes (per core, B=32, T=100, D=784, G=256, H=128):
  - all weights are bf16 on device: fp32 matmuls lower to 2x (HI/LO)
    LDWEIGHTS+MATMUL pairs on trn2, bf16 is single-pass and gets
    fast-weight-load on full 128-col tiles. Verified numerically
    (argmax decisions unchanged, rel err ~6e-3 vs the fp32 reference).
  - per-iteration weights are packed host-side into two HBM blobs so
    each iteration issues 3 large contiguous DMAs (triple-buffered)
    instead of 8 strided ones. W1/W2 (contraction D=784) are
    zero-padded to 896 = 7*128 rows so every lhsT tile is [128, 128].
  - everything stays feature-major [feat, batch]; all MLP matmuls are
    weight-stationary with the tiny [*, 32] activations streaming.
  - sampling feature-major: pert^T = logits^T + (gum^T - 1e9*mem^T);
    2-stage argmax (DVE k-reduce -> PE transpose -> DVE row max), then
    one-hot = (pert == k-max) * winning-row-indicator via is_equal
    (no ties on this data, verified); st = hard + soft -
    stop_grad(soft) == hard numerically, so softmax is skipped.
  - the gumbel tensor is pre-transposed host-side with -1e9 in the
    896-pad rows so padded features never win the argmax; bg2 is
    folded into it (all other biases are zeros, asserted).
  - PSUM start=True clears has_written for the whole bank, so split
    accumulation groups (LSTM gates prefetch the h@Whh half an
    iteration early) put start=True only on their first matmul.
  - persistent PSUM tiles with once-zeroed dead regions (ps_lf, ps_a1,
    ps_tr) let each consumer be one full-size DVE/ACT op.
  - leaky relu (slope 0.2) = ScalarE Prelu (NOT Lrelu, whose alpha is
    ignored); sigmoid/tanh/parametric_relu share one ACT table set.
    Gate columns are permuted host-side to [i,f,o,g] so the three
    sigmoids are one activation call.
  - tiny keep-warm matmuls chained to serial stages feed the PE HAM
    activity monitor through activation-heavy windows.
"""
from contextlib import ExitStack

import numpy as np
import ml_dtypes

import concourse.bass as bass
import concourse.mybir as mybir
import concourse.tile as tile
from concourse.vector_clock import ScopedClock
from concourse.bass_utils import run_bass_kernel_spmd

F32 = mybir.dt.float32
BF16 = mybir.dt.bfloat16
ALU = mybir.AluOpType
ACTF = mybir.ActivationFunctionType

NCORES = 8
B = 32          # per-core batch
D = 784
DP = 896        # D zero-padded to 7*128 for full-width lhsT tiles
KC = 128        # k-chunk width over the (padded) D axis
TW = 16         # valid width of the 7th (tail) chunk: 784 - 6*128
NK = 7
G = 256
H = 128
T = 100
NEGBIG = -1.0e9

# blobA per-partition column layout (bf16), 128 partitions:
#   W1 section: 7 k-chunks x 784   (W1pad[k*128+p, n])
#   W2 section: 7 k-chunks x 256   (W2pad[k*128+p, g])
A_W1 = 0
A_W2 = NK * D                 # 5488
A_COLS = NK * D + NK * G      # 7280

# blobB per-partition column layout (bf16), 128 partitions:
B_WG2 = 0                     # 2 k-chunks x 784 (Wg2[k*128+p, n])
B_WG1 = 2 * D                 # 256  (Wg1[p, g])
B_W3 = B_WG1 + G              # 2 k-chunks x 128 (W3[k*128+p, j])
B_W4 = B_W3 + 2 * H           # 128
B_WF2 = B_W4 + H              # 10
B_COLS = B_WF2 + 10           # 2218

USE_LRELU_ACT = True          # leaky relu on ScalarE; False -> DVE stt


class _TileContextSplitDrain(tile.TileContext):
    """This walrus build rejects >1 sem-wait on the kernel-tail Drain;
    split the accumulated waits across several sequential drains."""

    def _drain_and_barrier(self, tick_clock, wait_clock):
        drain_inst = self.nc.sync.drain()
        wait_clock.add_sem_waits(
            drain_inst.ins, ScopedClock({None: tick_clock.global_clock}))
        si = drain_inst.ins.sync_info
        waits = list(si.on_wait or []) if si is not None else []
        if len(waits) > 1:
            si.on_wait = [waits[0]]
            for w in waits[1:]:
                d2 = self.nc.sync.drain()
                if d2.ins.sync_info is None:
                    d2.ins.sync_info = mybir.SyncInfo(on_wait=[w], on_update=[])
                else:
                    d2.ins.sync_info.on_wait = [w]
        self.nc.all_engine_barrier()
        assert self.sems is not None
        popped = self.nc._tile_sem_poison_stack.pop()
        assert popped is self._sem_poison
        self.nc.clear_and_free_semaphores(list(self.sems.allocated().values()))
        self.nc.all_engine_barrier()


def _split_multi_waits(nc, limit=1):
    """This walrus accepts only `limit` sem-waits per instruction; move the
    excess onto same-engine sequencer NOPs inserted immediately before."""
    import copy

    proto = nc.vector.isa(nc.isa.Opcode.NEURON_ISA_TPB_OPCODE_NOP, {}).ins
    nop_ctr = [0]

    def make_nop(engine, waits):
        nop = copy.deepcopy(proto)
        nop_ctr[0] += 1
        nop.name = f"waitnop-{nop_ctr[0]}"
        nop.engine = engine
        nop.sync_info = mybir.SyncInfo(on_wait=list(waits), on_update=[])
        return nop

    skip = ("InstAllEngineBarrier", "InstEventSemaphore")
    for fn in nc.m.functions:
        for bb in fn.blocks:
            insts = bb.instructions
            if insts and insts[-1] is proto:
                insts.pop()
            out = []
            for inst in insts:
                si = inst.sync_info
                waits = list(si.on_wait or []) if si is not None else []
                if len(waits) > limit and type(inst).__name__ not in skip:
                    for i in range(0, len(waits) - limit, limit):
                        out.append(make_nop(inst.engine, waits[i:i + limit]))
                    si.on_wait = waits[len(waits) - limit:]
                out.append(inst)
            bb.instructions[:] = out


def _declare_params(nc):
    dp = nc.declare_dram_parameter
    p = {}
    p["x"] = dp("x", [B, D], F32, isOutput=False)
    # gumbel noise pre-transposed feature-major [T, KC, NK, B]; the dead
    # pad region [TW:, 6, :] is filled with -1e9 host-side
    p["gum"] = dp("gum", [T, KC, NK * B], F32, isOutput=False)
    p["blobA"] = dp("blobA", [T, 128, A_COLS], BF16, isOutput=False)
    p["blobB"] = dp("blobB", [T, 128, B_COLS], BF16, isOutput=False)
    p["Wih"] = dp("Wih", [H, 4 * H], BF16, isOutput=False)
    p["Whh"] = dp("Whh", [H, 4 * H], BF16, isOutput=False)
    p["ident"] = dp("ident", [B, B], F32, isOutput=False)
    p["ident128"] = dp("ident128", [128, 128], F32, isOutput=False)
    p["out"] = dp("out", [10, T * B], F32, isOutput=True)
    return p


def _leaky(nc, out_ap, in_ap, tmp_ap=None):
    """out = leaky_relu(in_, slope 0.2)."""
    if USE_LRELU_ACT:
        nc.scalar.activation(out_ap, in_ap, ACTF.Prelu, alpha=0.2)
    else:
        nc.vector.scalar_tensor_tensor(out_ap, in_ap, 0.2, in_ap,
                                       ALU.mult, ALU.max)


def _build(ctx, tc, p, w_bufs=3):
    nc = tc.nc

    const_pool = ctx.enter_context(tc.tile_pool(name="const", bufs=1))
    state_pool = ctx.enter_context(tc.tile_pool(name="state", bufs=1))
    wpool = ctx.enter_context(tc.tile_pool(name="w", bufs=w_bufs))
    psum = ctx.enter_context(tc.tile_pool(name="ps", bufs=1, space="PSUM"))

    WihS = const_pool.tile([H, 4 * H], BF16, tag="wih")
    WhhS = const_pool.tile([H, 4 * H], BF16, tag="whh")
    IDENT = const_pool.tile([B, B], F32, tag="ident")
    IDENT128 = const_pool.tile([128, 128], F32, tag="id128")
    XB = const_pool.tile([B, D], F32, tag="xb")
    nc.sync.dma_start(WihS[:], p["Wih"].ap())
    nc.sync.dma_start(WhhS[:], p["Whh"].ap())
    nc.sync.dma_start(IDENT[:], p["ident"].ap())
    nc.sync.dma_start(IDENT128[:], p["ident128"].ap())
    nc.sync.dma_start(XB[:], p["x"].ap())

    A_fm = state_pool.tile([H, B], BF16, tag="a")        # lin^T (bf16)
    H_fm = state_pool.tile([H, B], BF16, tag="h")
    C_fm = state_pool.tile([H, B], F32, tag="c")
    XFM = state_pool.tile([KC, NK, B], F32, tag="xfm")   # x^T (7x128 chunks)
    YT = state_pool.tile([KC, NK, B], BF16, tag="yt")    # (mem*x)^T bf16
    MEMFM = state_pool.tile([KC, NK, B], F32, tag="memfm")  # mask^T
    GMF = state_pool.tile([KC, NK, B], F32, tag="gmf")   # gum^T - 1e9*mem^T
    PERT = state_pool.tile([KC, NK, B], F32, tag="pert")  # pert^T
    TMX = state_pool.tile([KC, B], F32, tag="tmx")       # max over k
    MAXV = state_pool.tile([B, 1], F32, tag="maxv")      # global max
    E1 = state_pool.tile([B, KC], F32, tag="e1")         # argmax chunk-row
    EQ2 = state_pool.tile([KC, NK, B], F32, tag="eq2")
    HDF = state_pool.tile([KC, NK, B], F32, tag="hdf")   # one-hot fm
    HXF = state_pool.tile([KC, NK, B], BF16, tag="hxf")  # one-hot * x fm
    SIFO = state_pool.tile([H, 3, B], F32, tag="sifo")   # sig(i), sig(f), sig(o)
    TG = state_pool.tile([H, B], F32, tag="tg")
    TC = state_pool.tile([H, B], F32, tag="tc")
    U0 = state_pool.tile([H, B], F32, tag="u0")
    U1 = state_pool.tile([H, B], F32, tag="u1")
    G1 = state_pool.tile([H, 2, B], BF16, tag="g1")
    A1 = state_pool.tile([128, NK, B], BF16, tag="a1")   # f1 layer1, m-tiled
    A2 = state_pool.tile([H, 2, B], BF16, tag="a2")
    A3 = state_pool.tile([H, B], BF16, tag="a3")
    SOUT = state_pool.tile([10, T, B], F32, tag="sout")

    nc.vector.memset(A_fm[:], 0.0)
    nc.vector.memset(H_fm[:], 0.0)
    nc.vector.memset(C_fm[:], 0.0)
    nc.vector.memset(YT[:], 0.0)
    nc.vector.memset(MEMFM[:], 0.0)
    nc.vector.memset(XFM[:], 0.0)
    # A1/YT/XFM chunk 6 is only 16 partitions valid; zero the dead regions
    # once so the zero-padded W1/W2 k-chunk 6 contracts against zeros.
    nc.vector.memset(A1[:], 0.0)

    # x^T feature-major (7 PE transposes of [B,128] chunks, 16-wide tail).
    # ps_tr persists across iterations: its dead region [TW:, 6, :] is
    # zeroed once so the per-iteration y^T update is a single DVE op.
    ps_tr = psum.tile([KC, NK, B], F32, tag="tr")
    nc.vector.memset(ps_tr[:, 6, :], 0.0)
    for k in range(NK):
        kw = KC if k < 6 else TW
        nc.tensor.transpose(ps_tr[0:kw, k, :],
                            XB[:, k * KC:k * KC + kw], IDENT[:])
    nc.scalar.copy(XFM[:, 0:6, :], ps_tr[:, 0:6, :])
    nc.scalar.copy(XFM[0:TW, 6, :], ps_tr[0:TW, 6, :])

    # persistent PSUM tiles: fm logits and f1-layer-1 accumulators (dead
    # tail regions zeroed once so each consumer is a single full-size op)
    # and the scratch bank for the TMX transpose + keep-warm dummies
    ps_lf = psum.tile([KC, NK, B], F32, tag="lf")
    nc.vector.memset(ps_lf[:, 6, :], 0.0)
    ps_a1 = psum.tile([128, NK, B], F32, tag="a1")
    nc.vector.memset(ps_a1[:, 6, :], 0.0)
    ps_warm = psum.tile([B, 256], F32, tag="warm")

    # prologue: Whh half of iteration 0's gates (h starts at zero).
    # NOTE start=True clears has_written for the WHOLE bank, so it may
    # appear only on the first matmul of the 8-matmul gate group.
    ps_g = psum.tile([H, 4, B], F32, tag="g")
    for j in range(4):
        nc.tensor.matmul(ps_g[:, j, :], WhhS[:, j * H:(j + 1) * H],
                         H_fm[:], start=(j == 0), stop=False,
                         skip_group_check=True)

    for t in range(T):
        # ---- weight/noise loads for iteration t (3 contiguous DMAs)
        WA = wpool.tile([128, A_COLS], BF16, tag="wa")
        WB = wpool.tile([128, B_COLS], BF16, tag="wb")
        GUM = wpool.tile([KC, NK, B], F32, tag="gum")
        nc.sync.dma_start(WA[:], p["blobA"].ap()[t])
        nc.sync.dma_start(WB[:], p["blobB"].ap()[t])
        nc.sync.dma_start(
            GUM[:], p["gum"].ap()[t].rearrange("p (k b) -> p k b", k=NK))

        # fm masked gumbel noise (off critical path; DVE FIFO slot falls
        # in the gating/logits window of iteration t-1)
        nc.vector.scalar_tensor_tensor(GMF[:], MEMFM[:], NEGBIG, GUM[:],
                                       ALU.mult, ALU.add)

        # keep-warm dummies: tiny matmuls chained to each serial stage
        # keep the PE HAM clock gate fed through the activation-heavy
        # window. Garbage lands in ps_warm cols 128+.
        def _warm(col, rhs_ap, lhs_ap=None):
            nc.tensor.matmul(ps_warm[0:B, col:col + rhs_ap.shape[-1]],
                             lhs_ap if lhs_ap is not None else XFM[:, 0, :],
                             rhs_ap, start=True, stop=True)

        # ---- LSTM cell: finish gates = lin @ Wih + (h @ Whh, prefetched)
        # host permutes gate columns to [i, f, o, g]
        for j in range(4):
            nc.tensor.matmul(ps_g[:, j, :], WihS[:, j * H:(j + 1) * H],
                             A_fm[:], start=False, stop=(j == 3),
                             skip_group_check=True)
        nc.scalar.activation(SIFO[:], ps_g[:, 0:3, :], ACTF.Sigmoid)
        _warm(0, SIFO[:, 0, :])
        nc.scalar.activation(TG[:], ps_g[:, 3, :], ACTF.Tanh)
        _warm(32, TG[:])
        nc.vector.tensor_tensor(U0[:], SIFO[:, 1, :], C_fm[:], ALU.mult)
        nc.vector.tensor_tensor(U1[:], SIFO[:, 0, :], TG[:], ALU.mult)
        nc.vector.tensor_tensor(C_fm[:], U0[:], U1[:], ALU.add)
        _warm(64, C_fm[:])
        nc.scalar.activation(TC[:], C_fm[:], ACTF.Tanh)
        _warm(96, TC[:])
        nc.vector.tensor_tensor(H_fm[:], SIFO[:, 2, :], TC[:], ALU.mult)

        # prefetch the Whh half of the NEXT iteration's gates
        ps_g = psum.tile([H, 4, B], F32, tag="g")
        for j in range(4):
            nc.tensor.matmul(ps_g[:, j, :], WhhS[:, j * H:(j + 1) * H],
                             H_fm[:], start=(j == 0), stop=False,
                             skip_group_check=True)

        # ---- gating MLP: g1 = leaky(h @ Wg1); logits = g1 @ Wg2
        ps_sm = psum.tile([H, 5, B], F32, tag="sm")
        for m in range(2):
            nc.tensor.matmul(ps_sm[:, 3 + m, :],
                             WB[:, B_WG1 + m * H:B_WG1 + (m + 1) * H],
                             H_fm[:], start=True, stop=True)
        # logits feature-major, weight-stationary FWL tiles. ps_lf's dead
        # region [TW:, 6, :] stays zero from the init memset; start=True
        # appears only on the first matmul of the bank's group. G1's
        # leaky is split per k-chunk so the k=0 matmuls start before the
        # second chunk's activation finishes.
        for k in range(2):
            _leaky(nc, G1[:, k, :], ps_sm[:, 3 + k, :])
            for m in range(NK):
                mw = 128 if m < 6 else TW
                nc.tensor.matmul(ps_lf[0:mw, m, :],
                                 WB[:, k * D + m * 128:k * D + m * 128 + mw],
                                 G1[:, k, :],
                                 start=(k == 0 and m == 0),
                                 stop=(k == 1 and m == NK - 1),
                                 skip_group_check=True)

        # ---- sampling (feature-major): pert^T, 2-stage max, one-hot
        nc.vector.tensor_tensor(PERT[:], ps_lf[:], GMF[:], ALU.add)
        nc.vector.tensor_reduce(TMX[:], PERT[:].rearrange("p k b -> p b k"),
                                axis=mybir.AxisListType.X, op=ALU.max)
        nc.tensor.transpose(ps_warm[0:B, 0:KC], TMX[:], IDENT128[:])
        nc.vector.tensor_reduce(MAXV[:], ps_warm[0:B, 0:KC],
                                axis=mybir.AxisListType.X, op=ALU.max)
        nc.vector.tensor_scalar(E1[:], ps_warm[0:B, 0:KC], MAXV[:, 0:1],
                                None, ALU.is_equal)
        nc.tensor.transpose(ps_tr[:, 0, :], E1[:], IDENT[:])
        # one-hot fm = (pert == per-row k-max) * winning-row indicator
        nc.vector.tensor_tensor(
            EQ2[:], PERT[:],
            TMX[:].unsqueeze(1).broadcast_to((KC, NK, B)), ALU.is_equal)
        nc.vector.tensor_tensor(
            HDF[:], EQ2[:],
            ps_tr[:, 0, :].unsqueeze(1).broadcast_to((KC, NK, B)), ALU.mult)
        nc.vector.tensor_tensor(HXF[:], HDF[:], XFM[:], ALU.mult)
        nc.vector.tensor_tensor(YT[:], HXF[:], YT[:], ALU.add)
        # mask update (consumed next iteration; DVE slot falls in the
        # f1-MLP window)
        nc.vector.tensor_tensor(MEMFM[:], HDF[:], MEMFM[:], ALU.add)

        # ---- f1 MLP layer 1: a1 = leaky(y @ W1), m-tiles of 128 (last 16;
        # its dead rows stay zero from the init memset, and leaky(0)=0)
        for m in range(NK):
            mw = 128 if m < 6 else TW
            for k in range(NK):
                nc.tensor.matmul(ps_a1[0:mw, m, :],
                                 WA[:, A_W1 + k * D + m * 128:
                                    A_W1 + k * D + m * 128 + mw],
                                 YT[:, k, :],
                                 start=(k == 0), stop=(k == NK - 1))
        _leaky(nc, A1[:], ps_a1[:])

        # ---- layer 2: a2 = leaky(a1 @ W2), k-chunks follow A1 m-tiling
        ps_a2 = psum.tile([H, 2, B], F32, tag="a2")
        for m in range(2):
            for k in range(NK):
                nc.tensor.matmul(ps_a2[:, m, :],
                                 WA[:, A_W2 + k * G + m * H:
                                    A_W2 + k * G + (m + 1) * H],
                                 A1[:, k, :], start=(k == 0), stop=(k == NK - 1))
        # ---- layers 3/4 + classifier (reuses ps_sm banks 0..2); A2's
        # leaky is split per k-chunk so W3's first matmul starts earlier
        for k in range(2):
            _leaky(nc, A2[:, k, :], ps_a2[:, k, :])
            nc.tensor.matmul(ps_sm[:, 0, :],
                             WB[:, B_W3 + k * H:B_W3 + (k + 1) * H],
                             A2[:, k, :], start=(k == 0), stop=(k == 1))
        _leaky(nc, A3[:], ps_sm[:, 0, :])
        nc.tensor.matmul(ps_sm[:, 1, :], WB[:, B_W4:B_W4 + H], A3[:],
                         start=True, stop=True)
        _leaky(nc, A_fm[:], ps_sm[:, 1, :])
        nc.tensor.matmul(ps_sm[0:10, 2, :], WB[:, B_WF2:B_WF2 + 10], A_fm[:],
                         start=True, stop=True)
        # on DVE: keeps the ACT queue free for the next LSTM sigmoids
        nc.vector.tensor_copy(SOUT[:, t, :], ps_sm[0:10, 2, :])

    nc.sync.dma_start(p["out"].ap(), SOUT[:].rearrange("c t b -> c (t b)"))


_CACHE = {}
TRACE = False
LAST_RES = None


def _get_nc(w_bufs=3):
    key = ("nc", w_bufs)
    if key not in _CACHE:
        nc = bass.Bass("TRN2", target_bir_lowering=False, debug=False)
        p = _declare_params(nc)
        with _TileContextSplitDrain(nc) as tc:
            with ExitStack() as ctx:
                _build(ctx, tc, p, w_bufs=w_bufs)
        _split_multi_waits(nc)
        _CACHE[key] = nc
    return _CACHE[key]


def _pack_blobs(f):
    """Host-side weight packing into the two per-iteration DMA blobs."""
    bf = ml_dtypes.bfloat16
    W1 = f("W1")          # [T, 784, 784]
    W2 = f("W2")          # [T, 784, 256]
    W1p = np.zeros((T, DP, D), np.float32)
    W1p[:, :D] = W1
    W2p = np.zeros((T, DP, G), np.float32)
    W2p[:, :D] = W2
    a1 = W1p.reshape(T, NK, 128, D).transpose(0, 2, 1, 3).reshape(T, 128, NK * D)
    a2 = W2p.reshape(T, NK, 128, G).transpose(0, 2, 1, 3).reshape(T, 128, NK * G)
    blobA = np.ascontiguousarray(
        np.concatenate([a1, a2], axis=2)).astype(bf)

    Wg2 = f("Wg2")        # [T, 256, 784]
    Wg1 = f("Wg1")        # [T, 128, 256]
    W3 = f("W3")          # [T, 256, 128]
    W4 = f("W4")          # [T, 128, 128]
    Wf2 = f("Wf2")        # [T, 128, 10]
    b1 = Wg2.reshape(T, 2, 128, D).transpose(0, 2, 1, 3).reshape(T, 128, 2 * D)
    b3 = W3.reshape(T, 2, 128, H).transpose(0, 2, 1, 3).reshape(T, 128, 2 * H)
    blobB = np.ascontiguousarray(np.concatenate(
        [b1, Wg1, b3, W4, Wf2], axis=2)).astype(bf)
    return blobA, blobB


def kernel(**inputs) -> np.ndarray:
    f = lambda k: np.ascontiguousarray(np.asarray(inputs[k]), dtype=np.float32)
    x = f("x")
    gumbel = f("gumbel")
    bg2 = f("bg2")
    gum_all = gumbel + bg2[:, None, :]          # fold bg2 into the noise
    # remaining biases are zeros in this problem; verify cheaply
    for bn in ("b1", "b2", "b3", "b4", "bf2", "bg1", "bih", "bhh"):
        if bn in inputs and np.any(np.asarray(inputs[bn])):
            raise NotImplementedError(f"nonzero bias {bn} not supported")

    blobA, blobB = _pack_blobs(f)
    bf = ml_dtypes.bfloat16
    # permute gate blocks from torch order [i,f,g,o] to [i,f,o,g] so the
    # three sigmoids are one contiguous activation call
    perm = np.r_[0:H, H:2 * H, 3 * H:4 * H, 2 * H:3 * H]
    shared = {
        "blobA": blobA,
        "blobB": blobB,
        "Wih": np.ascontiguousarray(f("Wih")[:, perm]).astype(bf),
        "Whh": np.ascontiguousarray(f("Whh")[:, perm]).astype(bf),
        "ident": np.eye(B, dtype=np.float32),
        "ident128": np.eye(128, dtype=np.float32),
    }

    in_maps = []
    for c in range(NCORES):
        sl = slice(c * B, (c + 1) * B)
        m = dict(shared)
        m["x"] = np.ascontiguousarray(x[sl])
        # gumbel noise feature-major [T, KC, NK, B]; dead pad rows get
        # -1e9 so padded features can never win the argmax
        gc = gum_all[:, sl]                     # [T, B, D]
        gf = np.full((T, KC, NK, B), NEGBIG, np.float32)
        for k in range(NK):
            kw = KC if k < 6 else TW
            gf[:, 0:kw, k, :] = gc[:, :, k * KC:k * KC + kw].transpose(0, 2, 1)
        m["gum"] = gf.reshape(T, KC, NK * B)
        in_maps.append(m)

    nc = _get_nc()
    global LAST_RES
    res = run_bass_kernel_spmd(nc, in_maps, list(range(NCORES)), trace=TRACE)
    LAST_RES = res
    # per-core out is [10, T*B] feature-major; reassemble to [B, 10, T]
    outs = []
    for c in range(NCORES):
        o = res.results[c]["out"].reshape(10, T, B)
        outs.append(np.ascontiguousarray(o.transpose(2, 0, 1)))
    return np.concatenate(outs, axis=0).astype(np.float32)


# revision 49
# speedup vs baseline: 3.8094x; 1.0147x over previous
"""Trainium2 Bass kernel for nn_Network_81862076662591 (sampling network).

Self-contained: takes FULL inputs (as produced by the problem's
setup_inputs), data-parallel shards batch B=256 over 8 NeuronCores
(32 rows each, per-iteration weights replicated), runs the fused
LSTM + gumbel-argmax sampling + MLP scan on-device, and returns the
full [256, 10, 100] output.

Design notes (per core, B=32, T=100, D=784, G=256, H=128):
  - all weights are bf16 on device: fp32 matmuls lower to 2x (HI/LO)
    LDWEIGHTS+MATMUL pairs on trn2, bf16 is single-pass and gets
    fast-weight-load on full 128-col tiles. Verified numerically
    (argmax decisions unchanged, rel err ~6e-3 vs the fp32 reference).
  - per-iteration weights are packed host-side into two HBM blobs so
    each iteration issues 3 large contiguous DMAs (triple-buffered)
    instead of 8 strided ones. W1/W2 (contraction D=784) are
    zero-padded to 896 = 7*128 rows so every lhsT tile is [128, 128].
  - everything stays feature-major [feat, batch]; all MLP matmuls are
    weight-stationary with the tiny [*, 32] activations streaming.
  - sampling feature-major: pert^T = logits^T + (gum^T - 1e9*mem^T);
    2-stage argmax (DVE k-reduce -> PE transpose -> DVE row max), then
    one-hot = (pert == k-max) * winning-row-indicator via is_equal
    (no ties on this data, verified); st = hard + soft -
    stop_grad(soft) == hard numerically, so softmax is skipped.
  - the gumbel tensor is pre-transposed host-side with -1e9 in the
    896-pad rows so padded features never win the argmax; bg2 is
    folded into it (all other biases are zeros, asserted).
  - PSUM start=True clears has_written for the whole bank, so split
    accumulation groups (LSTM gates prefetch the h@Whh half an
    iteration early) put start=True only on their first matmul.
  - persistent PSUM tiles with once-zeroed dead regions (ps_lf, ps_a1,
    ps_tr) let each consumer be one full-size DVE/ACT op.
  - leaky relu (slope 0.2) = ScalarE Prelu (NOT Lrelu, whose alpha is
    ignored); sigmoid/tanh/parametric_relu share one ACT table set.
    Gate columns are permuted host-side to [i,f,o,g] so the three
    sigmoids are one activation call.
  - tiny keep-warm matmuls chained to serial stages feed the PE HAM
    activity monitor through activation-heavy windows.
"""
from contextlib import ExitStack

import numpy as np
import ml_dtypes

import concourse.bass as bass
import concourse.mybir as mybir
import concourse.tile as tile
from concourse.vector_clock import ScopedClock
from concourse.bass_utils import run_bass_kernel_spmd

F32 = mybir.dt.float32
BF16 = mybir.dt.bfloat16
ALU = mybir.AluOpType
ACTF = mybir.ActivationFunctionType

NCORES = 8
B = 32          # per-core batch
D = 784
DP = 896        # D zero-padded to 7*128 for full-width lhsT tiles
KC = 128        # k-chunk width over the (padded) D axis
TW = 16         # valid width of the 7th (tail) chunk: 784 - 6*128
NK = 7
G = 256
H = 128
T = 100
NEGBIG = -1.0e9

# blobA per-partition column layout (bf16), 128 partitions:
#   W1 section: 7 k-chunks x 784   (W1pad[k*128+p, n])
#   W2 section: 7 k-chunks x 256   (W2pad[k*128+p, g])
A_W1 = 0
A_W2 = NK * D                 # 5488
A_COLS = NK * D + NK * G      # 7280

# blobB per-partition column layout (bf16), 128 partitions:
B_WG2 = 0                     # 2 k-chunks x 784 (Wg2[k*128+p, n])
B_WG1 = 2 * D                 # 256  (Wg1[p, g])
B_W3 = B_WG1 + G              # 2 k-chunks x 128 (W3[k*128+p, j])
B_W4 = B_W3 + 2 * H           # 128
B_WF2 = B_W4 + H              # 10
B_COLS = B_WF2 + 10           # 2218

USE_LRELU_ACT = True          # leaky relu on ScalarE; False -> DVE stt


class _TileContextSplitDrain(tile.TileContext):
    """This walrus build rejects >1 sem-wait on the kernel-tail Drain;
    split the accumulated waits across several sequential drains."""

    def _drain_and_barrier(self, tick_clock, wait_clock):
        drain_inst = self.nc.sync.drain()
        wait_clock.add_sem_waits(
            drain_inst.ins, ScopedClock({None: tick_clock.global_clock}))
        si = drain_inst.ins.sync_info
        waits = list(si.on_wait or []) if si is not None else []
        if len(waits) > 1:
            si.on_wait = [waits[0]]
            for w in waits[1:]:
                d2 = self.nc.sync.drain()
                if d2.ins.sync_info is None:
                    d2.ins.sync_info = mybir.SyncInfo(on_wait=[w], on_update=[])
                else:
                    d2.ins.sync_info.on_wait = [w]
        self.nc.all_engine_barrier()
        assert self.sems is not None
        popped = self.nc._tile_sem_poison_stack.pop()
        assert popped is self._sem_poison
        self.nc.clear_and_free_semaphores(list(self.sems.allocated().values()))
        self.nc.all_engine_barrier()


def _split_multi_waits(nc, limit=1):
    """This walrus accepts only `limit` sem-waits per instruction; move the
    excess onto same-engine sequencer NOPs inserted immediately before."""
    import copy

    proto = nc.vector.isa(nc.isa.Opcode.NEURON_ISA_TPB_OPCODE_NOP, {}).ins
    nop_ctr = [0]

    def make_nop(engine, waits):
        nop = copy.deepcopy(proto)
        nop_ctr[0] += 1
        nop.name = f"waitnop-{nop_ctr[0]}"
        nop.engine = engine
        nop.sync_info = mybir.SyncInfo(on_wait=list(waits), on_update=[])
        return nop

    skip = ("InstAllEngineBarrier", "InstEventSemaphore")
    for fn in nc.m.functions:
        for bb in fn.blocks:
            insts = bb.instructions
            if insts and insts[-1] is proto:
                insts.pop()
            out = []
            for inst in insts:
                si = inst.sync_info
                waits = list(si.on_wait or []) if si is not None else []
                if len(waits) > limit and type(inst).__name__ not in skip:
                    for i in range(0, len(waits) - limit, limit):
                        out.append(make_nop(inst.engine, waits[i:i + limit]))
                    si.on_wait = waits[len(waits) - limit:]
                out.append(inst)
            bb.instructions[:] = out


def _declare_params(nc):
    dp = nc.declare_dram_parameter
    p = {}
    p["x"] = dp("x", [B, D], F32, isOutput=False)
    # gumbel noise pre-transposed feature-major [T, KC, NK, B]; the dead
    # pad region [TW:, 6, :] is filled with -1e9 host-side
    p["gum"] = dp("gum", [T, KC, NK * B], F32, isOutput=False)
    p["blobA"] = dp("blobA", [T, 128, A_COLS], BF16, isOutput=False)
    p["blobB"] = dp("blobB", [T, 128, B_COLS], BF16, isOutput=False)
    p["Wih"] = dp("Wih", [H, 4 * H], BF16, isOutput=False)
    p["Whh"] = dp("Whh", [H, 4 * H], BF16, isOutput=False)
    p["ident"] = dp("ident", [B, B], F32, isOutput=False)
    p["ident128"] = dp("ident128", [128, 128], F32, isOutput=False)
    p["out"] = dp("out", [10, T * B], F32, isOutput=True)
    return p


def _leaky(nc, out_ap, in_ap, tmp_ap=None):
    """out = leaky_relu(in_, slope 0.2)."""
    if USE_LRELU_ACT:
        nc.scalar.activation(out_ap, in_ap, ACTF.Prelu, alpha=0.2)
    else:
        nc.vector.scalar_tensor_tensor(out_ap, in_ap, 0.2, in_ap,
                                       ALU.mult, ALU.max)


def _build(ctx, tc, p, w_bufs=3):
    nc = tc.nc

    const_pool = ctx.enter_context(tc.tile_pool(name="const", bufs=1))
    state_pool = ctx.enter_context(tc.tile_pool(name="state", bufs=1))
    wpool = ctx.enter_context(tc.tile_pool(name="w", bufs=w_bufs))
    psum = ctx.enter_context(tc.tile_pool(name="ps", bufs=1, space="PSUM"))

    WihS = const_pool.tile([H, 4 * H], BF16, tag="wih")
    WhhS = const_pool.tile([H, 4 * H], BF16, tag="whh")
    IDENT = const_pool.tile([B, B], F32, tag="ident")
    IDENT128 = const_pool.tile([128, 128], F32, tag="id128")
    XB = const_pool.tile([B, D], F32, tag="xb")
    nc.sync.dma_start(WihS[:], p["Wih"].ap())
    nc.sync.dma_start(WhhS[:], p["Whh"].ap())
    nc.sync.dma_start(IDENT[:], p["ident"].ap())
    nc.sync.dma_start(IDENT128[:], p["ident128"].ap())
    nc.sync.dma_start(XB[:], p["x"].ap())

    A_fm = state_pool.tile([H, B], BF16, tag="a")        # lin^T (bf16)
    H_fm = state_pool.tile([H, B], BF16, tag="h")
    XFM = state_pool.tile([KC, NK, B], F32, tag="xfm")   # x^T (7x128 chunks)
    YT = state_pool.tile([KC, NK, B], BF16, tag="yt")    # (mem*x)^T bf16
    MEMFM = state_pool.tile([KC, NK, B], F32, tag="memfm")  # mask^T
    GMF = state_pool.tile([KC, NK, B], F32, tag="gmf")   # gum^T - 1e9*mem^T
    # pert^T stored [KC, B, NK] so the k-max reduce reads contiguously
    PERT = state_pool.tile([KC, B, NK], F32, tag="pert")
    TMX = state_pool.tile([KC, B], F32, tag="tmx")       # max over k
    MAXV = state_pool.tile([B, 1], F32, tag="maxv")      # global max
    E1 = state_pool.tile([B, KC], F32, tag="e1")         # argmax chunk-row
    EQ2 = state_pool.tile([KC, NK, B], F32, tag="eq2")
    HDF = state_pool.tile([KC, NK, B], F32, tag="hdf")   # one-hot fm
    HXF = state_pool.tile([KC, NK, B], BF16, tag="hxf")  # one-hot * x fm
    SIFO = state_pool.tile([H, 3, B], F32, tag="sifo")   # sig(i), sig(f), sig(o)
    # TGC packs tanh(g) (slot 0) next to the persistent c state (slot 1)
    # so both LSTM gate multiplies are ONE tensor_tensor against SIFO
    TGC = state_pool.tile([H, 2, B], F32, tag="tgc")
    TC = state_pool.tile([H, B], F32, tag="tc")
    U01 = state_pool.tile([H, 2, B], F32, tag="u01")
    G1 = state_pool.tile([H, 2, B], BF16, tag="g1")
    A1 = state_pool.tile([128, NK, B], BF16, tag="a1")   # f1 layer1, m-tiled
    A2 = state_pool.tile([H, 2, B], BF16, tag="a2")
    A3 = state_pool.tile([H, B], BF16, tag="a3")
    SOUT = state_pool.tile([10, T, B], F32, tag="sout")

    nc.vector.memset(A_fm[:], 0.0)
    nc.vector.memset(H_fm[:], 0.0)
    nc.vector.memset(TGC[:], 0.0)
    nc.vector.memset(YT[:], 0.0)
    nc.vector.memset(MEMFM[:], 0.0)
    nc.vector.memset(XFM[:], 0.0)
    # A1/YT/XFM chunk 6 is only 16 partitions valid; zero the dead regions
    # once so the zero-padded W1/W2 k-chunk 6 contracts against zeros.
    nc.vector.memset(A1[:], 0.0)

    # x^T feature-major (7 PE transposes of [B,128] chunks, 16-wide tail).
    # ps_tr persists across iterations: its dead region [TW:, 6, :] is
    # zeroed once so the per-iteration y^T update is a single DVE op.
    ps_tr = psum.tile([KC, NK, B], F32, tag="tr")
    nc.vector.memset(ps_tr[:, 6, :], 0.0)
    for k in range(NK):
        kw = KC if k < 6 else TW
        nc.tensor.transpose(ps_tr[0:kw, k, :],
                            XB[:, k * KC:k * KC + kw], IDENT[:])
    nc.scalar.copy(XFM[:, 0:6, :], ps_tr[:, 0:6, :])
    nc.scalar.copy(XFM[0:TW, 6, :], ps_tr[0:TW, 6, :])

    # persistent PSUM tiles: fm logits and f1-layer-1 accumulators (dead
    # tail regions zeroed once so each consumer is a single full-size op)
    # and the scratch bank for the TMX transpose + keep-warm dummies
    ps_lf = psum.tile([KC, NK, B], F32, tag="lf")
    nc.vector.memset(ps_lf[:, 6, :], 0.0)
    ps_a1 = psum.tile([128, NK, B], F32, tag="a1")
    nc.vector.memset(ps_a1[:, 6, :], 0.0)
    ps_warm = psum.tile([B, 256], F32, tag="warm")

    # prologue: Whh half of iteration 0's gates (h starts at zero).
    # NOTE start=True clears has_written for the WHOLE bank, so it may
    # appear only on the first matmul of the 8-matmul gate group.
    ps_g = psum.tile([H, 4, B], F32, tag="g")
    for j in range(4):
        nc.tensor.matmul(ps_g[:, j, :], WhhS[:, j * H:(j + 1) * H],
                         H_fm[:], start=(j == 0), stop=False,
                         skip_group_check=True)

    for t in range(T):
        # ---- weight/noise loads for iteration t (3 contiguous DMAs)
        WA = wpool.tile([128, A_COLS], BF16, tag="wa")
        WB = wpool.tile([128, B_COLS], BF16, tag="wb")
        GUM = wpool.tile([KC, NK, B], F32, tag="gum")
        nc.sync.dma_start(WA[:], p["blobA"].ap()[t])
        nc.sync.dma_start(WB[:], p["blobB"].ap()[t])
        nc.sync.dma_start(
            GUM[:], p["gum"].ap()[t].rearrange("p (k b) -> p k b", k=NK))

        # fm masked gumbel noise (off critical path; DVE FIFO slot falls
        # in the gating/logits window of iteration t-1)
        nc.vector.scalar_tensor_tensor(GMF[:], MEMFM[:], NEGBIG, GUM[:],
                                       ALU.mult, ALU.add)

        # keep-warm dummies: tiny matmuls chained to each serial stage
        # keep the PE HAM clock gate fed through the activation-heavy
        # window. Garbage lands in ps_warm cols 128+.
        def _warm(col, rhs_ap, lhs_ap=None):
            nc.tensor.matmul(ps_warm[0:B, col:col + rhs_ap.shape[-1]],
                             lhs_ap if lhs_ap is not None else XFM[:, 0, :],
                             rhs_ap, start=True, stop=True)

        # ---- LSTM cell: finish gates = lin @ Wih + (h @ Whh, prefetched)
        # host permutes gate columns to [i, f, o, g]
        for j in range(4):
            nc.tensor.matmul(ps_g[:, j, :], WihS[:, j * H:(j + 1) * H],
                             A_fm[:], start=False, stop=(j == 3),
                             skip_group_check=True)
        nc.scalar.activation(SIFO[:], ps_g[:, 0:3, :], ACTF.Sigmoid)
        _warm(0, SIFO[:, 0, :])
        nc.scalar.activation(TGC[:, 0, :], ps_g[:, 3, :], ACTF.Tanh)
        _warm(32, TGC[:, 0, :])
        # one op: [i*tanh(g), f*c] (SIFO slots [i, f]; TGC [tanh(g), c])
        nc.vector.tensor_tensor(U01[:], SIFO[:, 0:2, :], TGC[:], ALU.mult)
        nc.vector.tensor_tensor(TGC[:, 1, :], U01[:, 0, :], U01[:, 1, :],
                                ALU.add)
        _warm(64, TGC[:, 1, :])
        nc.scalar.activation(TC[:], TGC[:, 1, :], ACTF.Tanh)
        _warm(96, TC[:])
        nc.vector.tensor_tensor(H_fm[:], SIFO[:, 2, :], TC[:], ALU.mult)

        # prefetch the Whh half of the NEXT iteration's gates
        ps_g = psum.tile([H, 4, B], F32, tag="g")
        for j in range(4):
            nc.tensor.matmul(ps_g[:, j, :], WhhS[:, j * H:(j + 1) * H],
                             H_fm[:], start=(j == 0), stop=False,
                             skip_group_check=True)

        # ---- gating MLP: g1 = leaky(h @ Wg1); logits = g1 @ Wg2
        ps_sm = psum.tile([H, 5, B], F32, tag="sm")
        for m in range(2):
            nc.tensor.matmul(ps_sm[:, 3 + m, :],
                             WB[:, B_WG1 + m * H:B_WG1 + (m + 1) * H],
                             H_fm[:], start=True, stop=True)
        _leaky(nc, G1[:], ps_sm[:, 3:5, :])
        # logits feature-major, weight-stationary FWL tiles. ps_lf's dead
        # region [TW:, 6, :] stays zero from the init memset; start=True
        # appears only on the first matmul of the bank's group.
        for k in range(2):
            for m in range(NK):
                mw = 128 if m < 6 else TW
                nc.tensor.matmul(ps_lf[0:mw, m, :],
                                 WB[:, k * D + m * 128:k * D + m * 128 + mw],
                                 G1[:, k, :],
                                 start=(k == 0 and m == 0),
                                 stop=(k == 1 and m == NK - 1),
                                 skip_group_check=True)

        # ---- sampling (feature-major): pert^T, 2-stage max, one-hot
        nc.vector.tensor_tensor(PERT[:].rearrange("p b k -> p k b"),
                                ps_lf[:], GMF[:], ALU.add)
        nc.vector.tensor_reduce(TMX[:], PERT[:],
                                axis=mybir.AxisListType.X, op=ALU.max)
        nc.tensor.transpose(ps_warm[0:B, 0:KC], TMX[:], IDENT128[:])
        nc.vector.tensor_reduce(MAXV[:], ps_warm[0:B, 0:KC],
                                axis=mybir.AxisListType.X, op=ALU.max)
        nc.vector.tensor_scalar(E1[:], ps_warm[0:B, 0:KC], MAXV[:, 0:1],
                                None, ALU.is_equal)
        nc.tensor.transpose(ps_tr[:, 0, :], E1[:], IDENT[:])
        # one-hot fm = (pert == per-row k-max) * winning-row indicator
        nc.vector.tensor_tensor(
            EQ2[:], PERT[:].rearrange("p b k -> p k b"),
            TMX[:].unsqueeze(1).broadcast_to((KC, NK, B)), ALU.is_equal)
        nc.vector.tensor_tensor(
            HDF[:], EQ2[:],
            ps_tr[:, 0, :].unsqueeze(1).broadcast_to((KC, NK, B)), ALU.mult)
        nc.vector.tensor_tensor(HXF[:], HDF[:], XFM[:], ALU.mult)
        nc.vector.tensor_tensor(YT[:], HXF[:], YT[:], ALU.add)
        # mask update (consumed next iteration; DVE slot falls in the
        # f1-MLP window)
        nc.vector.tensor_tensor(MEMFM[:], HDF[:], MEMFM[:], ALU.add)

        # ---- f1 MLP layer 1: a1 = leaky(y @ W1), m-tiles of 128 (last 16;
        # its dead rows stay zero from the init memset, and leaky(0)=0)
        for m in range(NK):
            mw = 128 if m < 6 else TW
            for k in range(NK):
                nc.tensor.matmul(ps_a1[0:mw, m, :],
                                 WA[:, A_W1 + k * D + m * 128:
                                    A_W1 + k * D + m * 128 + mw],
                                 YT[:, k, :],
                                 start=(k == 0), stop=(k == NK - 1))
        _leaky(nc, A1[:], ps_a1[:])

        # ---- layer 2: a2 = leaky(a1 @ W2), k-chunks follow A1 m-tiling
        ps_a2 = psum.tile([H, 2, B], F32, tag="a2")
        for m in range(2):
            for k in range(NK):
                nc.tensor.matmul(ps_a2[:, m, :],
                                 WA[:, A_W2 + k * G + m * H:
                                    A_W2 + k * G + (m + 1) * H],
                                 A1[:, k, :], start=(k == 0), stop=(k == NK - 1))
        _leaky(nc, A2[:], ps_a2[:])

        # ---- layers 3/4 + classifier (reuses ps_sm banks 0..2)
        for k in range(2):
            nc.tensor.matmul(ps_sm[:, 0, :],
                             WB[:, B_W3 + k * H:B_W3 + (k + 1) * H],
                             A2[:, k, :], start=(k == 0), stop=(k == 1))
        _leaky(nc, A3[:], ps_sm[:, 0, :])
        nc.tensor.matmul(ps_sm[:, 1, :], WB[:, B_W4:B_W4 + H], A3[:],
                         start=True, stop=True)
        _leaky(nc, A_fm[:], ps_sm[:, 1, :])
        nc.tensor.matmul(ps_sm[0:10, 2, :], WB[:, B_WF2:B_WF2 + 10], A_fm[:],
                         start=True, stop=True)
        # on DVE: keeps the ACT queue free for the next LSTM sigmoids
        nc.vector.tensor_copy(SOUT[:, t, :], ps_sm[0:10, 2, :])

    nc.sync.dma_start(p["out"].ap(), SOUT[:].rearrange("c t b -> c (t b)"))


_CACHE = {}
TRACE = False
LAST_RES = None


def _get_nc(w_bufs=3):
    key = ("nc", w_bufs)
    if key not in _CACHE:
        nc = bass.Bass("TRN2", target_bir_lowering=False, debug=False)
        p = _declare_params(nc)
        with _TileContextSplitDrain(nc) as tc:
            with ExitStack() as ctx:
                _build(ctx, tc, p, w_bufs=w_bufs)
        _split_multi_waits(nc)
        _CACHE[key] = nc
    return _CACHE[key]


def _pack_blobs(f):
    """Host-side weight packing into the two per-iteration DMA blobs."""
    bf = ml_dtypes.bfloat16
    W1 = f("W1")          # [T, 784, 784]
    W2 = f("W2")          # [T, 784, 256]
    W1p = np.zeros((T, DP, D), np.float32)
    W1p[:, :D] = W1
    W2p = np.zeros((T, DP, G), np.float32)
    W2p[:, :D] = W2
    a1 = W1p.reshape(T, NK, 128, D).transpose(0, 2, 1, 3).reshape(T, 128, NK * D)
    a2 = W2p.reshape(T, NK, 128, G).transpose(0, 2, 1, 3).reshape(T, 128, NK * G)
    blobA = np.ascontiguousarray(
        np.concatenate([a1, a2], axis=2)).astype(bf)

    Wg2 = f("Wg2")        # [T, 256, 784]
    Wg1 = f("Wg1")        # [T, 128, 256]
    W3 = f("W3")          # [T, 256, 128]
    W4 = f("W4")          # [T, 128, 128]
    Wf2 = f("Wf2")        # [T, 128, 10]
    b1 = Wg2.reshape(T, 2, 128, D).transpose(0, 2, 1, 3).reshape(T, 128, 2 * D)
    b3 = W3.reshape(T, 2, 128, H).transpose(0, 2, 1, 3).reshape(T, 128, 2 * H)
    blobB = np.ascontiguousarray(np.concatenate(
        [b1, Wg1, b3, W4, Wf2], axis=2)).astype(bf)
    return blobA, blobB


def kernel(**inputs) -> np.ndarray:
    f = lambda k: np.ascontiguousarray(np.asarray(inputs[k]), dtype=np.float32)
    x = f("x")
    gumbel = f("gumbel")
    bg2 = f("bg2")
    gum_all = gumbel + bg2[:, None, :]          # fold bg2 into the noise
    # remaining biases are zeros in this problem; verify cheaply
    for bn in ("b1", "b2", "b3", "b4", "bf2", "bg1", "bih", "bhh"):
        if bn in inputs and np.any(np.asarray(inputs[bn])):
            raise NotImplementedError(f"nonzero bias {bn} not supported")

    blobA, blobB = _pack_blobs(f)
    bf = ml_dtypes.bfloat16
    # permute gate blocks from torch order [i,f,g,o] to [i,f,o,g] so the
    # three sigmoids are one contiguous activation call
    perm = np.r_[0:H, H:2 * H, 3 * H:4 * H, 2 * H:3 * H]
    shared = {
        "blobA": blobA,
        "blobB": blobB,
        "Wih": np.ascontiguousarray(f("Wih")[:, perm]).astype(bf),
        "Whh": np.ascontiguousarray(f("Whh")[:, perm]).astype(bf),
        "ident": np.eye(B, dtype=np.float32),
        "ident128": np.eye(128, dtype=np.float32),
    }

    in_maps = []
    for c in range(NCORES):
        sl = slice(c * B, (c + 1) * B)
        m = dict(shared)
        m["x"] = np.ascontiguousarray(x[sl])
        # gumbel noise feature-major [T, KC, NK, B]; dead pad rows get
        # -1e9 so padded features can never win the argmax
        gc = gum_all[:, sl]                     # [T, B, D]
        gf = np.full((T, KC, NK, B), NEGBIG, np.float32)
        for k in range(NK):
            kw = KC if k < 6 else TW
            gf[:, 0:kw, k, :] = gc[:, :, k * KC:k * KC + kw].transpose(0, 2, 1)
        m["gum"] = gf.reshape(T, KC, NK * B)
        in_maps.append(m)

    nc = _get_nc()
    global LAST_RES
    res = run_bass_kernel_spmd(nc, in_maps, list(range(NCORES)), trace=TRACE)
    LAST_RES = res
    # per-core out is [10, T*B] feature-major; reassemble to [B, 10, T]
    outs = []
    for c in range(NCORES):
        o = res.results[c]["out"].reshape(10, T, B)
        outs.append(np.ascontiguousarray(o.transpose(2, 0, 1)))
    return np.concatenate(outs, axis=0).astype(np.float32)


# revision 53
# speedup vs baseline: 3.8670x; 1.0151x over previous
"""Trainium2 Bass kernel for nn_Network_81862076662591 (sampling network).

Self-contained: takes FULL inputs (as produced by the problem's
setup_inputs), data-parallel shards batch B=256 over 8 NeuronCores
(32 rows each, per-iteration weights replicated), runs the fused
LSTM + gumbel-argmax sampling + MLP scan on-device, and returns the
full [256, 10, 100] output.

Design notes (per core, B=32, T=100, D=784, G=256, H=128):
  - all weights are bf16 on device: fp32 matmuls lower to 2x (HI/LO)
    LDWEIGHTS+MATMUL pairs on trn2, bf16 is single-pass and gets
    fast-weight-load on full 128-col tiles. Verified numerically
    (argmax decisions unchanged, rel err ~6e-3 vs the fp32 reference).
  - per-iteration weights are packed host-side into two HBM blobs so
    each iteration issues 3 large contiguous DMAs (triple-buffered)
    instead of 8 strided ones. W1/W2 (contraction D=784) are
    zero-padded to 896 = 7*128 rows so every lhsT tile is [128, 128].
  - everything stays feature-major [feat, batch]; all MLP matmuls are
    weight-stationary with the tiny [*, 32] activations streaming.
  - sampling feature-major: pert^T = logits^T + (gum^T - 1e9*mem^T);
    2-stage argmax (DVE k-reduce -> PE transpose -> DVE row max), then
    one-hot = (pert == k-max) * winning-row-indicator via is_equal
    (no ties on this data, verified); st = hard + soft -
    stop_grad(soft) == hard numerically, so softmax is skipped.
  - the gumbel tensor is pre-transposed host-side with -1e9 in the
    896-pad rows so padded features never win the argmax; bg2 is
    folded into it (all other biases are zeros, asserted).
  - PSUM start=True clears has_written for the whole bank, so split
    accumulation groups (LSTM gates prefetch the h@Whh half an
    iteration early) put start=True only on their first matmul.
  - persistent PSUM tiles with once-zeroed dead regions (ps_lf, ps_a1,
    ps_tr) let each consumer be one full-size DVE/ACT op.
  - leaky relu (slope 0.2) = ScalarE Prelu (NOT Lrelu, whose alpha is
    ignored); sigmoid/tanh/parametric_relu share one ACT table set.
    Gate columns are permuted host-side to [i,f,o,g] so the three
    sigmoids are one activation call.
  - tiny keep-warm matmuls chained to serial stages feed the PE HAM
    activity monitor through activation-heavy windows.
"""
from contextlib import ExitStack

import numpy as np
import ml_dtypes

import concourse.bass as bass
import concourse.mybir as mybir
import concourse.tile as tile
from concourse.vector_clock import ScopedClock
from concourse.bass_utils import run_bass_kernel_spmd

F32 = mybir.dt.float32
BF16 = mybir.dt.bfloat16
ALU = mybir.AluOpType
ACTF = mybir.ActivationFunctionType

NCORES = 8
B = 32          # per-core batch
D = 784
DP = 896        # D zero-padded to 7*128 for full-width lhsT tiles
KC = 128        # k-chunk width over the (padded) D axis
TW = 16         # valid width of the 7th (tail) chunk: 784 - 6*128
NK = 7
G = 256
H = 128
T = 100
NEGBIG = -1.0e9

# blobA per-partition column layout (bf16), 128 partitions:
#   W1 section: 7 k-chunks x 784   (W1pad[k*128+p, n])
#   W2 section: 7 k-chunks x 256   (W2pad[k*128+p, g])
A_W1 = 0
A_W2 = NK * D                 # 5488
A_COLS = NK * D + NK * G      # 7280

# blobB per-partition column layout (bf16), 128 partitions:
B_WG2 = 0                     # 2 k-chunks x 784 (Wg2[k*128+p, n])
B_WG1 = 2 * D                 # 256  (Wg1[p, g])
B_W3 = B_WG1 + G              # 2 k-chunks x 128 (W3[k*128+p, j])
B_W4 = B_W3 + 2 * H           # 128
B_WF2 = B_W4 + H              # 10
B_COLS = B_WF2 + 10           # 2218

USE_LRELU_ACT = True          # leaky relu on ScalarE; False -> DVE stt


class _TileContextSplitDrain(tile.TileContext):
    """This walrus build rejects >1 sem-wait on the kernel-tail Drain;
    split the accumulated waits across several sequential drains."""

    def _drain_and_barrier(self, tick_clock, wait_clock):
        drain_inst = self.nc.sync.drain()
        wait_clock.add_sem_waits(
            drain_inst.ins, ScopedClock({None: tick_clock.global_clock}))
        si = drain_inst.ins.sync_info
        waits = list(si.on_wait or []) if si is not None else []
        if len(waits) > 1:
            si.on_wait = [waits[0]]
            for w in waits[1:]:
                d2 = self.nc.sync.drain()
                if d2.ins.sync_info is None:
                    d2.ins.sync_info = mybir.SyncInfo(on_wait=[w], on_update=[])
                else:
                    d2.ins.sync_info.on_wait = [w]
        self.nc.all_engine_barrier()
        assert self.sems is not None
        popped = self.nc._tile_sem_poison_stack.pop()
        assert popped is self._sem_poison
        self.nc.clear_and_free_semaphores(list(self.sems.allocated().values()))
        self.nc.all_engine_barrier()


def _split_multi_waits(nc, limit=1):
    """This walrus accepts only `limit` sem-waits per instruction; move the
    excess onto same-engine sequencer NOPs inserted immediately before."""
    import copy

    proto = nc.vector.isa(nc.isa.Opcode.NEURON_ISA_TPB_OPCODE_NOP, {}).ins
    nop_ctr = [0]

    def make_nop(engine, waits):
        nop = copy.deepcopy(proto)
        nop_ctr[0] += 1
        nop.name = f"waitnop-{nop_ctr[0]}"
        nop.engine = engine
        nop.sync_info = mybir.SyncInfo(on_wait=list(waits), on_update=[])
        return nop

    skip = ("InstAllEngineBarrier", "InstEventSemaphore")
    for fn in nc.m.functions:
        for bb in fn.blocks:
            insts = bb.instructions
            if insts and insts[-1] is proto:
                insts.pop()
            out = []
            for inst in insts:
                si = inst.sync_info
                waits = list(si.on_wait or []) if si is not None else []
                if len(waits) > limit and type(inst).__name__ not in skip:
                    for i in range(0, len(waits) - limit, limit):
                        out.append(make_nop(inst.engine, waits[i:i + limit]))
                    si.on_wait = waits[len(waits) - limit:]
                out.append(inst)
            bb.instructions[:] = out


def _declare_params(nc):
    dp = nc.declare_dram_parameter
    p = {}
    p["x"] = dp("x", [B, D], F32, isOutput=False)
    # gumbel noise pre-transposed feature-major [T, KC, NK, B]; the dead
    # pad region [TW:, 6, :] is filled with -1e9 host-side
    p["gum"] = dp("gum", [T, KC, NK * B], F32, isOutput=False)
    p["blobA"] = dp("blobA", [T, 128, A_COLS], BF16, isOutput=False)
    p["blobB"] = dp("blobB", [T, 128, B_COLS], BF16, isOutput=False)
    p["Wih"] = dp("Wih", [H, 4 * H], BF16, isOutput=False)
    p["Whh"] = dp("Whh", [H, 4 * H], BF16, isOutput=False)
    p["ident"] = dp("ident", [B, B], F32, isOutput=False)
    p["ident128"] = dp("ident128", [128, 128], F32, isOutput=False)
    p["out"] = dp("out", [10, T * B], F32, isOutput=True)
    return p


def _leaky(nc, out_ap, in_ap, tmp_ap=None):
    """out = leaky_relu(in_, slope 0.2)."""
    if USE_LRELU_ACT:
        nc.scalar.activation(out_ap, in_ap, ACTF.Prelu, alpha=0.2)
    else:
        nc.vector.scalar_tensor_tensor(out_ap, in_ap, 0.2, in_ap,
                                       ALU.mult, ALU.max)


def _build(ctx, tc, p, w_bufs=3):
    nc = tc.nc

    const_pool = ctx.enter_context(tc.tile_pool(name="const", bufs=1))
    state_pool = ctx.enter_context(tc.tile_pool(name="state", bufs=1))
    wpool = ctx.enter_context(tc.tile_pool(name="w", bufs=w_bufs))
    psum = ctx.enter_context(tc.tile_pool(name="ps", bufs=1, space="PSUM"))

    WihS = const_pool.tile([H, 4 * H], BF16, tag="wih")
    WhhS = const_pool.tile([H, 4 * H], BF16, tag="whh")
    IDENT = const_pool.tile([B, B], F32, tag="ident")
    IDENT128 = const_pool.tile([128, 128], F32, tag="id128")
    XB = const_pool.tile([B, D], F32, tag="xb")
    nc.sync.dma_start(WihS[:], p["Wih"].ap())
    nc.sync.dma_start(WhhS[:], p["Whh"].ap())
    nc.sync.dma_start(IDENT[:], p["ident"].ap())
    nc.sync.dma_start(IDENT128[:], p["ident128"].ap())
    nc.sync.dma_start(XB[:], p["x"].ap())

    A_fm = state_pool.tile([H, B], BF16, tag="a")        # lin^T (bf16)
    H_fm = state_pool.tile([H, B], BF16, tag="h")
    C_fm = state_pool.tile([H, B], F32, tag="c")
    # x^T bf16: y is bf16 anyway, and 16-bit inputs put the one-hot*x
    # multiply in the DVE 2x mode
    XFM = state_pool.tile([KC, NK, B], BF16, tag="xfm")
    YT = state_pool.tile([KC, NK, B], BF16, tag="yt")    # (mem*x)^T bf16
    MEMFM = state_pool.tile([KC, NK, B], F32, tag="memfm")  # mask^T
    GMF = state_pool.tile([KC, NK, B], F32, tag="gmf")   # gum^T - 1e9*mem^T
    PERT = state_pool.tile([KC, NK, B], F32, tag="pert")  # pert^T
    TMX = state_pool.tile([KC, B], F32, tag="tmx")       # max over k
    MAXV = state_pool.tile([B, 1], F32, tag="maxv")      # global max
    E1 = state_pool.tile([B, KC], F32, tag="e1")         # argmax chunk-row
    EQ2 = state_pool.tile([KC, NK, B], F32, tag="eq2")
    HDF = state_pool.tile([KC, NK, B], BF16, tag="hdf")  # one-hot fm (exact)
    HXF = state_pool.tile([KC, NK, B], BF16, tag="hxf")  # one-hot * x fm
    SIFO = state_pool.tile([H, 3, B], F32, tag="sifo")   # sig(i), sig(f), sig(o)
    TG = state_pool.tile([H, B], F32, tag="tg")
    TC = state_pool.tile([H, B], F32, tag="tc")
    U0 = state_pool.tile([H, B], F32, tag="u0")
    U1 = state_pool.tile([H, B], F32, tag="u1")
    G1 = state_pool.tile([H, 2, B], BF16, tag="g1")
    A1 = state_pool.tile([128, NK, B], BF16, tag="a1")   # f1 layer1, m-tiled
    A2 = state_pool.tile([H, 2, B], BF16, tag="a2")
    A3 = state_pool.tile([H, B], BF16, tag="a3")
    SOUT = state_pool.tile([10, T, B], F32, tag="sout")

    nc.vector.memset(A_fm[:], 0.0)
    nc.vector.memset(H_fm[:], 0.0)
    nc.vector.memset(C_fm[:], 0.0)
    nc.vector.memset(YT[:], 0.0)
    nc.vector.memset(MEMFM[:], 0.0)
    nc.vector.memset(XFM[:], 0.0)
    # A1/YT/XFM chunk 6 is only 16 partitions valid; zero the dead regions
    # once so the zero-padded W1/W2 k-chunk 6 contracts against zeros.
    nc.vector.memset(A1[:], 0.0)

    # x^T feature-major (7 PE transposes of [B,128] chunks, 16-wide tail).
    # ps_tr persists across iterations: its dead region [TW:, 6, :] is
    # zeroed once so the per-iteration y^T update is a single DVE op.
    ps_tr = psum.tile([KC, NK, B], F32, tag="tr")
    nc.vector.memset(ps_tr[:, 6, :], 0.0)
    for k in range(NK):
        kw = KC if k < 6 else TW
        nc.tensor.transpose(ps_tr[0:kw, k, :],
                            XB[:, k * KC:k * KC + kw], IDENT[:])
    nc.scalar.copy(XFM[:, 0:6, :], ps_tr[:, 0:6, :])
    nc.scalar.copy(XFM[0:TW, 6, :], ps_tr[0:TW, 6, :])

    # persistent PSUM tiles: fm logits and f1-layer-1 accumulators (dead
    # tail regions zeroed once so each consumer is a single full-size op)
    # and the scratch bank for the TMX transpose + keep-warm dummies
    ps_lf = psum.tile([KC, NK, B], F32, tag="lf")
    nc.vector.memset(ps_lf[:, 6, :], 0.0)
    ps_a1 = psum.tile([128, NK, B], F32, tag="a1")
    nc.vector.memset(ps_a1[:, 6, :], 0.0)
    ps_warm = psum.tile([B, 256], F32, tag="warm")

    # prologue: Whh half of iteration 0's gates (h starts at zero).
    # NOTE start=True clears has_written for the WHOLE bank, so it may
    # appear only on the first matmul of the 8-matmul gate group.
    ps_g = psum.tile([H, 4, B], F32, tag="g")
    for j in range(4):
        nc.tensor.matmul(ps_g[:, j, :], WhhS[:, j * H:(j + 1) * H],
                         H_fm[:], start=(j == 0), stop=False,
                         skip_group_check=True)

    for t in range(T):
        # ---- weight/noise loads for iteration t (3 contiguous DMAs)
        WA = wpool.tile([128, A_COLS], BF16, tag="wa")
        WB = wpool.tile([128, B_COLS], BF16, tag="wb")
        GUM = wpool.tile([KC, NK, B], F32, tag="gum")
        nc.sync.dma_start(WA[:], p["blobA"].ap()[t])
        nc.sync.dma_start(WB[:], p["blobB"].ap()[t])
        nc.sync.dma_start(
            GUM[:], p["gum"].ap()[t].rearrange("p (k b) -> p k b", k=NK))

        # fm masked gumbel noise (off critical path; DVE FIFO slot falls
        # in the gating/logits window of iteration t-1)
        nc.vector.scalar_tensor_tensor(GMF[:], MEMFM[:], NEGBIG, GUM[:],
                                       ALU.mult, ALU.add)

        # keep-warm dummies: tiny matmuls chained to each serial stage
        # keep the PE HAM clock gate fed through the activation-heavy
        # window. Garbage lands in ps_warm cols 128+.
        def _warm(col, rhs_ap, lhs_ap=None):
            nc.tensor.matmul(ps_warm[0:B, col:col + rhs_ap.shape[-1]],
                             lhs_ap if lhs_ap is not None else
                             IDENT128[:, 0:32],
                             rhs_ap, start=True, stop=True)

        # ---- LSTM cell: finish gates = lin @ Wih + (h @ Whh, prefetched)
        # host permutes gate columns to [i, f, o, g]
        for j in range(4):
            nc.tensor.matmul(ps_g[:, j, :], WihS[:, j * H:(j + 1) * H],
                             A_fm[:], start=False, stop=(j == 3),
                             skip_group_check=True)
        nc.scalar.activation(SIFO[:], ps_g[:, 0:3, :], ACTF.Sigmoid)
        _warm(0, SIFO[:, 0, :])
        nc.scalar.activation(TG[:], ps_g[:, 3, :], ACTF.Tanh)
        _warm(32, TG[:])
        nc.vector.tensor_tensor(U0[:], SIFO[:, 1, :], C_fm[:], ALU.mult)
        nc.vector.tensor_tensor(U1[:], SIFO[:, 0, :], TG[:], ALU.mult)
        nc.vector.tensor_tensor(C_fm[:], U0[:], U1[:], ALU.add)
        _warm(64, C_fm[:])
        nc.scalar.activation(TC[:], C_fm[:], ACTF.Tanh)
        _warm(96, TC[:])
        nc.vector.tensor_tensor(H_fm[:], SIFO[:, 2, :], TC[:], ALU.mult)

        # prefetch the Whh half of the NEXT iteration's gates
        ps_g = psum.tile([H, 4, B], F32, tag="g")
        for j in range(4):
            nc.tensor.matmul(ps_g[:, j, :], WhhS[:, j * H:(j + 1) * H],
                             H_fm[:], start=(j == 0), stop=False,
                             skip_group_check=True)

        # ---- gating MLP: g1 = leaky(h @ Wg1); logits = g1 @ Wg2
        ps_sm = psum.tile([H, 5, B], F32, tag="sm")
        for m in range(2):
            nc.tensor.matmul(ps_sm[:, 3 + m, :],
                             WB[:, B_WG1 + m * H:B_WG1 + (m + 1) * H],
                             H_fm[:], start=True, stop=True)
        _leaky(nc, G1[:], ps_sm[:, 3:5, :])
        # logits feature-major, weight-stationary FWL tiles. ps_lf's dead
        # region [TW:, 6, :] stays zero from the init memset; start=True
        # appears only on the first matmul of the bank's group.
        for k in range(2):
            for m in range(NK):
                mw = 128 if m < 6 else TW
                nc.tensor.matmul(ps_lf[0:mw, m, :],
                                 WB[:, k * D + m * 128:k * D + m * 128 + mw],
                                 G1[:, k, :],
                                 start=(k == 0 and m == 0),
                                 stop=(k == 1 and m == NK - 1),
                                 skip_group_check=True)

        # ---- sampling (feature-major): pert^T, 2-stage max, one-hot
        nc.vector.tensor_tensor(PERT[:], ps_lf[:], GMF[:], ALU.add)
        nc.vector.tensor_reduce(TMX[:], PERT[:].rearrange("p k b -> p b k"),
                                axis=mybir.AxisListType.X, op=ALU.max)
        nc.tensor.transpose(ps_warm[0:B, 0:KC], TMX[:], IDENT128[:])
        nc.vector.tensor_reduce(MAXV[:], ps_warm[0:B, 0:KC],
                                axis=mybir.AxisListType.X, op=ALU.max)
        nc.vector.tensor_scalar(E1[:], ps_warm[0:B, 0:KC], MAXV[:, 0:1],
                                None, ALU.is_equal)
        nc.tensor.transpose(ps_tr[:, 0, :], E1[:], IDENT[:])
        # one-hot fm = (pert == per-row k-max) * winning-row indicator
        nc.vector.tensor_tensor(
            EQ2[:], PERT[:],
            TMX[:].unsqueeze(1).broadcast_to((KC, NK, B)), ALU.is_equal)
        nc.vector.tensor_tensor(
            HDF[:], EQ2[:],
            ps_tr[:, 0, :].unsqueeze(1).broadcast_to((KC, NK, B)), ALU.mult)
        nc.vector.tensor_tensor(HXF[:], HDF[:], XFM[:], ALU.mult)
        nc.vector.tensor_tensor(YT[:], HXF[:], YT[:], ALU.add)
        # mask update (consumed next iteration; DVE slot falls in the
        # f1-MLP window)
        nc.vector.tensor_tensor(MEMFM[:], HDF[:], MEMFM[:], ALU.add)

        # ---- f1 MLP layer 1: a1 = leaky(y @ W1), m-tiles of 128 (last 16;
        # its dead rows stay zero from the init memset, and leaky(0)=0)
        for m in range(NK):
            mw = 128 if m < 6 else TW
            for k in range(NK):
                nc.tensor.matmul(ps_a1[0:mw, m, :],
                                 WA[:, A_W1 + k * D + m * 128:
                                    A_W1 + k * D + m * 128 + mw],
                                 YT[:, k, :],
                                 start=(k == 0), stop=(k == NK - 1))
        _leaky(nc, A1[:], ps_a1[:])

        # ---- layer 2: a2 = leaky(a1 @ W2), k-chunks follow A1 m-tiling
        ps_a2 = psum.tile([H, 2, B], F32, tag="a2")
        for m in range(2):
            for k in range(NK):
                nc.tensor.matmul(ps_a2[:, m, :],
                                 WA[:, A_W2 + k * G + m * H:
                                    A_W2 + k * G + (m + 1) * H],
                                 A1[:, k, :], start=(k == 0), stop=(k == NK - 1))
        _leaky(nc, A2[:], ps_a2[:])

        # ---- layers 3/4 + classifier (reuses ps_sm banks 0..2)
        for k in range(2):
            nc.tensor.matmul(ps_sm[:, 0, :],
                             WB[:, B_W3 + k * H:B_W3 + (k + 1) * H],
                             A2[:, k, :], start=(k == 0), stop=(k == 1))
        _leaky(nc, A3[:], ps_sm[:, 0, :])
        nc.tensor.matmul(ps_sm[:, 1, :], WB[:, B_W4:B_W4 + H], A3[:],
                         start=True, stop=True)
        _leaky(nc, A_fm[:], ps_sm[:, 1, :])
        nc.tensor.matmul(ps_sm[0:10, 2, :], WB[:, B_WF2:B_WF2 + 10], A_fm[:],
                         start=True, stop=True)
        # on DVE: keeps the ACT queue free for the next LSTM sigmoids
        nc.vector.tensor_copy(SOUT[:, t, :], ps_sm[0:10, 2, :])

    nc.sync.dma_start(p["out"].ap(), SOUT[:].rearrange("c t b -> c (t b)"))


_CACHE = {}
TRACE = False
LAST_RES = None


def _get_nc(w_bufs=3):
    key = ("nc", w_bufs)
    if key not in _CACHE:
        nc = bass.Bass("TRN2", target_bir_lowering=False, debug=False)
        p = _declare_params(nc)
        with _TileContextSplitDrain(nc) as tc:
            with ExitStack() as ctx:
                _build(ctx, tc, p, w_bufs=w_bufs)
        _split_multi_waits(nc)
        _CACHE[key] = nc
    return _CACHE[key]


def _pack_blobs(f):
    """Host-side weight packing into the two per-iteration DMA blobs."""
    bf = ml_dtypes.bfloat16
    W1 = f("W1")          # [T, 784, 784]
    W2 = f("W2")          # [T, 784, 256]
    W1p = np.zeros((T, DP, D), np.float32)
    W1p[:, :D] = W1
    W2p = np.zeros((T, DP, G), np.float32)
    W2p[:, :D] = W2
    a1 = W1p.reshape(T, NK, 128, D).transpose(0, 2, 1, 3).reshape(T, 128, NK * D)
    a2 = W2p.reshape(T, NK, 128, G).transpose(0, 2, 1, 3).reshape(T, 128, NK * G)
    blobA = np.ascontiguousarray(
        np.concatenate([a1, a2], axis=2)).astype(bf)

    Wg2 = f("Wg2")        # [T, 256, 784]
    Wg1 = f("Wg1")        # [T, 128, 256]
    W3 = f("W3")          # [T, 256, 128]
    W4 = f("W4")          # [T, 128, 128]
    Wf2 = f("Wf2")        # [T, 128, 10]
    b1 = Wg2.reshape(T, 2, 128, D).transpose(0, 2, 1, 3).reshape(T, 128, 2 * D)
    b3 = W3.reshape(T, 2, 128, H).transpose(0, 2, 1, 3).reshape(T, 128, 2 * H)
    blobB = np.ascontiguousarray(np.concatenate(
        [b1, Wg1, b3, W4, Wf2], axis=2)).astype(bf)
    return blobA, blobB


def kernel(**inputs) -> np.ndarray:
    f = lambda k: np.ascontiguousarray(np.asarray(inputs[k]), dtype=np.float32)
    x = f("x")
    gumbel = f("gumbel")
    bg2 = f("bg2")
    gum_all = gumbel + bg2[:, None, :]          # fold bg2 into the noise
    # remaining biases are zeros in this problem; verify cheaply
    for bn in ("b1", "b2", "b3", "b4", "bf2", "bg1", "bih", "bhh"):
        if bn in inputs and np.any(np.asarray(inputs[bn])):
            raise NotImplementedError(f"nonzero bias {bn} not supported")

    blobA, blobB = _pack_blobs(f)
    bf = ml_dtypes.bfloat16
    # permute gate blocks from torch order [i,f,g,o] to [i,f,o,g] so the
    # three sigmoids are one contiguous activation call
    perm = np.r_[0:H, H:2 * H, 3 * H:4 * H, 2 * H:3 * H]
    shared = {
        "blobA": blobA,
        "blobB": blobB,
        "Wih": np.ascontiguousarray(f("Wih")[:, perm]).astype(bf),
        "Whh": np.ascontiguousarray(f("Whh")[:, perm]).astype(bf),
        "ident": np.eye(B, dtype=np.float32),
        "ident128": np.eye(128, dtype=np.float32),
    }

    in_maps = []
    for c in range(NCORES):
        sl = slice(c * B, (c + 1) * B)
        m = dict(shared)
        m["x"] = np.ascontiguousarray(x[sl])
        # gumbel noise feature-major [T, KC, NK, B]; dead pad rows get
        # -1e9 so padded features can never win the argmax
        gc = gum_all[:, sl]                     # [T, B, D]
        gf = np.full((T, KC, NK, B), NEGBIG, np.float32)
        for k in range(NK):
            kw = KC if k < 6 else TW
            gf[:, 0:kw, k, :] = gc[:, :, k * KC:k * KC + kw].transpose(0, 2, 1)
        m["gum"] = gf.reshape(T, KC, NK * B)
        in_maps.append(m)

    nc = _get_nc()
    global LAST_RES
    res = run_bass_kernel_spmd(nc, in_maps, list(range(NCORES)), trace=TRACE)
    LAST_RES = res
    # per-core out is [10, T*B] feature-major; reassemble to [B, 10, T]
    outs = []
    for c in range(NCORES):
        o = res.results[c]["out"].reshape(10, T, B)
        outs.append(np.ascontiguousarray(o.transpose(2, 0, 1)))
    return np.concatenate(outs, axis=0).astype(np.float32)
